# revision 45
# baseline (speedup 1.0000x reference)
"""Trainium2 Bass kernel for nn_BackProjLayer.

Math: the reference computes, per sample n,
    eigh(S) -> (lam, V);  G = V @ diag(sqrt(max(lam,0)));  y = D^H G
    out[n,p] = sum_d |y[p,d]|^2 - tau[p] = [D^H S_plus D]_pp - tau[p]
Since S = A A^H / Nch is Hermitian PSD by construction, S_plus == S up to
float32 eigensolver noise, so no eigendecomposition is needed:
    out[n,p] = Re(d_p^H S[n] d_p) - tau[p]
With S = Sr + i Si (Sr sym, Si antisym) and d = dr + i di this is a real
bilinear form; by Hermitian symmetry it reduces to 16 features per sample
(4 diag Sr, 6 offdiag Sr doubled, 6 offdiag Si doubled):
    out = X16.T @ W16 - tau     X16 (16, N), W16 (16, 242)

Device kernel (default config, layout A + bf16split):
  - exact-fp32 matmul via a K-stacked bf16 3-way split: X16 = Xh+Xm+Xl,
    W16 = Wh+Wm+Wl (bf16 planes); the six significant cross-terms
    (hh, mh, hm, lh, hl, mm) plus three const-one rows against the -tau
    bf16 planes are stacked into ONE K=99 bf16 matmul per 128-sample tile
    (dropped ml/lm/ll terms are ~2^-27 relative).
  - per 128-sample tile: PSUM[128,242] = lhsT(99,128).T @ Wstack(99,242);
    PSUM->SBUF copies alternate scalar/vector engines; 8 grouped ~1MB
    output DMAs alternate the two HWDGE queues (sync/scalar).
  - built on Bacc + TileContext: Bacc.compile() splits multi-wait
    sync_infos into chained EventSemaphores (TRN2 ISA allows only one
    sync wait per instruction).

Sharding: pure data parallel over N across 8 cores (8192 samples/core);
host packs per-core inputs, device returns (8192, 242) per core,
host concatenates.
"""

import sys

for _p in ("/opt/trn_rl_repo", "/root/.axon_site/_ro/trn_rl_repo"):
    if _p not in sys.path:
        sys.path.insert(0, _p)

import numpy as np

N_SAMPLES = 65536
N_CH = 4
N_PX = 242
N_CORES = 8
N_LOC = N_SAMPLES // N_CORES  # 8192

K_FEAT = 2 * N_CH * N_CH + 1  # 33

TILE = 128
N_TILES = N_LOC // TILE  # 64

DEFAULT_CFG = dict(
    layout="A",      # A: samples-on-partitions; B: pixels-on-partitions
    bf16split=True,  # exact-fp32 via K-stacked bf16 3-way split (K=99)
    f32r=True,       # (non-split A only) float32r matmul
    pack=2,          # matmul outputs per PSUM tile / per copy instruction
    psum_bufs=4,     # PSUM pool slots
    groups=8,        # number of output DMAs
    dma_engines=("sync", "scalar"),  # round-robin for output DMAs
    copy_pattern="vvs",  # per-copy engine cycle: s=scalar, v=vector
    x_chunks=8,      # input DMA chunks
    slice_w=512,     # B: samples per matmul (moving dim)
    group_w=2048,    # B: samples per output DMA group
    in_eng="sync",   # engine issuing input DMAs
    linear_out=True, # B: write output groups as contiguous HBM blocks
    pmajor=True,     # A: partition-major sample mapping (contiguous 7.7KB
                     # HBM runs per partition in output DMAs)
    tail="sem_only", # lighter Tile exit: sem-only barrier around sem clears
)

# B layout constants
PX_CHUNKS = (128, 114)  # pixels per chunk (DMA uses 8 partitions/SDMA engine,
                        # so a 128-partition transfer engages all 16 engines)
W_PAD = 242          # leading cols of xw holding the stacked W planes
IDX_DIAG = [0, 5, 10, 15]       # S[c,c] positions in the c*4+c' flattening
IDX_OFF = [1, 2, 3, 6, 7, 11]   # S[c,c'] c<c' positions

_BUILT = {}


def _tc_class(base, cfg):
    """Optionally lighten the Tile kernel-tail: keep the drain (output DMA
    completion) and the semaphore clears (needed for NEFF re-execution),
    but trim barrier work per cfg['tail'] mode."""
    mode = cfg.get("tail", "full")
    if mode == "full":
        return base

    from concourse.vector_clock import ScopedClock

    class _TC(base):
        def _drain_and_barrier(self, tick_clock, wait_clock):
            nc = self.nc
            drain_inst = nc.sync.drain()
            wait_clock.add_sem_waits(
                drain_inst.ins, ScopedClock({None: tick_clock.global_clock})
            )
            if mode == "sem_only":
                nc.all_engine_barrier(sem_only=True)
            else:
                nc.all_engine_barrier()
            popped = nc._tile_sem_poison_stack.pop()
            assert popped is self._sem_poison
            nc.clear_and_free_semaphores(list(self.sems.allocated().values()))
            if mode not in ("no2nd", "sem_only"):
                nc.all_engine_barrier()

    return _TC


def _build_nc(cfg):
    import concourse.mybir as mybir
    from concourse import bacc
    from concourse.tile import TileContext

    f32 = mybir.dt.float32
    f32r = mybir.dt.float32r
    bf16 = mybir.dt.bfloat16

    if cfg.get("bf16split"):
        # exact-fp32 bf16 3-way split (see _build_nc_b docstring), with two
        # extra const-one rows pairing against the -tau bf16 planes: K=99
        pxp = N_PX
        in_dt = bf16
        kf = 99
    else:
        pxp = 256 if cfg["f32r"] else N_PX
        in_dt = f32r if cfg["f32r"] else f32
        kf = K_FEAT
    ps_stride = 256
    pack = cfg["pack"]
    groups = cfg["groups"]
    tiles_per_group = N_TILES // groups
    assert tiles_per_group % pack == 0 or pack % tiles_per_group == 0

    # Bacc (not plain Bass): its compile() lowers multi-wait sync_infos into
    # chained EventSemaphores (TRN2 allows 1 wait/instruction).
    nc = bacc.Bacc("TRN2", target_bir_lowering=False, debug=False)
    TileContext = _tc_class(TileContext, cfg)
    xTw = nc.declare_dram_parameter("xTw", [kf, pxp + N_LOC], in_dt, isOutput=False)
    out = nc.declare_dram_parameter("out", [N_LOC, N_PX], f32, isOutput=True)

    if cfg.get("pmajor"):
        # partition-major sample mapping: tile t, partition p <-> sample
        # n = p*64 + t. Each partition's 8-tile group lands in 8
        # CONSECUTIVE output rows -> 7744B contiguous HBM runs per
        # partition (vs 968B strided), much better SDMA descriptor
        # efficiency. Host permutes the input columns to match.
        out_g = out.rearrange(
            "(p g j) c -> g p (j c)", p=TILE, g=groups, j=tiles_per_group
        )
    else:
        out_g = out.rearrange("(g j p) c -> g p j c", p=TILE, j=tiles_per_group)
    x_chunk = N_LOC // cfg["x_chunks"]

    with TileContext(nc) as tc:
        with (
            tc.tile_pool(name="xin", bufs=1) as xpool,
            tc.tile_pool(name="ps", bufs=cfg["psum_bufs"], space="PSUM") as pspool,
            tc.tile_pool(name="ob", bufs=1) as opool,
        ):
            if cfg.get("sep_in"):
                in_eng = getattr(nc, cfg["in_eng"])
                wt_tile = xpool.tile([kf, pxp], in_dt, tag="w")
                in_eng.dma_start(wt_tile[:], xTw[:, :pxp])
                wt = wt_tile[:]
                xts = []
                for ci in range(cfg["x_chunks"]):
                    xt = xpool.tile([kf, x_chunk], in_dt, tag=f"x{ci}")
                    lo = pxp + ci * x_chunk
                    in_eng.dma_start(xt[:], xTw[:, lo : lo + x_chunk])
                    xts.append(xt)

                def lhs_ap(t):
                    ci, off = divmod(t * TILE, x_chunk)
                    return xts[ci][:, off : off + TILE]
            else:
                in_eng = getattr(nc, cfg["in_eng"])
                xt0 = xpool.tile([kf, pxp + N_LOC], in_dt)
                in_eng.dma_start(xt0[:, : pxp + x_chunk], xTw[:, : pxp + x_chunk])
                for ci in range(1, cfg["x_chunks"]):
                    lo = pxp + ci * x_chunk
                    in_eng.dma_start(xt0[:, lo : lo + x_chunk], xTw[:, lo : lo + x_chunk])
                wt = xt0[:, :pxp]

                def lhs_ap(t):
                    off = pxp + t * TILE
                    return xt0[:, off : off + TILE]

            copy_engines = {
                "s": nc.scalar.copy,
                "v": nc.vector.tensor_copy,
            }
            dma_engines = [getattr(nc, e) for e in cfg["dma_engines"]]

            copy_idx = 0
            for g in range(groups):
                gt = opool.tile([TILE, tiles_per_group * N_PX], f32, tag=f"g{g}")
                for jp in range(tiles_per_group // pack):
                    ps = pspool.tile([TILE, pack * ps_stride], f32)
                    for h in range(pack):
                        t = g * tiles_per_group + jp * pack + h
                        nc.tensor.matmul(
                            ps[:, h * ps_stride : h * ps_stride + pxp],
                            lhs_ap(t),
                            wt,
                            start=True,
                            stop=True,
                        )
                    src = ps[:].rearrange("p (h c) -> p h c", h=pack)[:, :, :N_PX]
                    lo = jp * pack * N_PX
                    dst = gt[:, lo : lo + pack * N_PX].rearrange(
                        "p (h c) -> p h c", h=pack
                    )
                    pat = cfg["copy_pattern"]
                    copy_engines[pat[copy_idx % len(pat)]](dst, src)
                    copy_idx += 1
                if cfg.get("pmajor"):
                    dma_engines[g % len(dma_engines)].dma_start(out_g[g], gt[:])
                else:
                    dma_engines[g % len(dma_engines)].dma_start(
                        out_g[g],
                        gt[:].rearrange("p (j c) -> p j c", j=tiles_per_group),
                    )

    nc.compile()
    return nc


def _build_nc_b(cfg):
    """Pixels-on-partitions layout with exact-fp32 bf16 3-way split.

    The bilinear form is reduced to 16 features per sample via the
    Hermitian symmetry of S (4 diag + 6 sym-offdiag + 6 antisym-offdiag,
    off-diagonal weights doubled). X16 = Xh+Xm+Xl, W16 = Wh+Wm+Wl (bf16
    planes); the six significant cross-terms (hh, mh, hm, lh, hl, mm) are
    K-stacked into ONE bf16 matmul with K = 6*16 = 96:
      [Xh;Xm;Xh;Xl;Xh;Xm] x [Wh;Wh;Wm;Wh;Wl;Wm]
    (dropped ml/lm/ll terms are ~2^-27 relative -> fp32-grade accuracy).
    Stationary operand is the W side (reloaded only on pixel-chunk switch),
    moving is samples, so the PE streams at 1 col/cycle. The output lands
    transposed (242 x 8192) and is unscrambled on the host. tau is applied
    per-partition during the PSUM->SBUF copy (ACT Identity bias / DVE
    tensor_scalar add). Pixel chunks are 128 + 114 so output DMAs engage
    all 16 SDMA engines (8 partitions each).
    """
    import concourse.mybir as mybir
    from concourse import bacc
    from concourse.tile import TileContext

    f32 = mybir.dt.float32
    bf16 = mybir.dt.bfloat16

    slice_w = cfg["slice_w"]
    group_w = cfg["group_w"]
    slices_per_group = group_w // slice_w
    n_groups = N_LOC // group_w

    nc = bacc.Bacc("TRN2", target_bir_lowering=False, debug=False)
    TileContext = _tc_class(TileContext, cfg)
    xw = nc.declare_dram_parameter("xw", [96, W_PAD + N_LOC], bf16, isOutput=False)
    taus = nc.declare_dram_parameter("taus", [128, 2], f32, isOutput=False)
    if cfg["linear_out"]:
        # each (px_w, group_w) staging tile lands as one contiguous HBM
        # block -> the DMA splits evenly across all 16 SDMA engines
        out_flat = nc.declare_dram_parameter("out_flat", [N_PX * N_LOC], f32, isOutput=True)
    else:
        outT = nc.declare_dram_parameter("outT", [N_PX, N_LOC], f32, isOutput=True)

    n_xchunks = cfg["x_chunks"]
    x_chunk = N_LOC // n_xchunks

    with TileContext(nc) as tc:
        with (
            tc.tile_pool(name="xin", bufs=1) as xpool,
            tc.tile_pool(name="ps", bufs=cfg["psum_bufs"], space="PSUM") as pspool,
            tc.tile_pool(name="ob", bufs=1) as opool,
        ):
            # W planes + tau first (small, fast), then per-chunk sample
            # tiles so early matmuls only wait on their own chunk's DMA
            in_eng = getattr(nc, cfg["in_eng"])
            wtile = xpool.tile([96, W_PAD], bf16, tag="w")
            in_eng.dma_start(wtile[:], xw[:, :W_PAD])
            tt = xpool.tile([128, 2], f32, tag="taus")
            in_eng.dma_start(tt[:], taus[:])
            xts = []
            for ci in range(n_xchunks):
                xt = xpool.tile([96, x_chunk], bf16, tag=f"x{ci}")
                lo = W_PAD + ci * x_chunk
                in_eng.dma_start(xt[:], xw[:, lo : lo + x_chunk])
                xts.append(xt)

            dma_engines = [getattr(nc, e) for e in cfg["dma_engines"]]
            copy_idx = 0
            dma_idx = 0
            px_lo = 0
            for c, px_w in enumerate(PX_CHUNKS):
                wa = wtile[:, px_lo : px_lo + px_w]
                for g in range(n_groups):
                    gt = opool.tile([px_w, group_w], f32, tag=f"g{c}_{g}")
                    for sj in range(slices_per_group):
                        s = g * slices_per_group + sj
                        ci, off = divmod(s * slice_w, x_chunk)
                        xm = xts[ci][:, off : off + slice_w]
                        ps = pspool.tile([px_w, slice_w], f32, tag="ps")
                        nc.tensor.matmul(ps[:], wa, xm, start=True, stop=True)
                        dst = gt[:, sj * slice_w : (sj + 1) * slice_w]
                        pat = cfg["copy_pattern"]
                        eng = pat[copy_idx % len(pat)]
                        if eng == "s":
                            nc.scalar.activation(
                                dst,
                                ps[:],
                                mybir.ActivationFunctionType.Identity,
                                bias=tt[:px_w, c : c + 1],
                            )
                        else:
                            nc.vector.tensor_scalar_add(
                                dst, ps[:], tt[:px_w, c : c + 1]
                            )
                        copy_idx += 1
                    if cfg["linear_out"]:
                        off = (px_lo * N_LOC) + g * px_w * group_w
                        dest = out_flat[off : off + px_w * group_w].rearrange(
                            "(p c) -> p c", p=px_w
                        )
                    else:
                        dest = outT[
                            px_lo : px_lo + px_w, g * group_w : (g + 1) * group_w
                        ]
                    dma_engines[dma_idx % len(dma_engines)].dma_start(dest, gt[:])
                    dma_idx += 1
                px_lo += px_w

    nc.compile()
    return nc


def _get_nc(cfg=None):
    cfg = dict(DEFAULT_CFG, **(cfg or {}))
    key = tuple(sorted((k, str(v)) for k, v in cfg.items()))
    if key not in _BUILT:
        builder = _build_nc_b if cfg["layout"] == "B" else _build_nc
        _BUILT[key] = (builder(cfg), cfg)
    return _BUILT[key]


def _pack_host(S_re, S_im, D_re, D_im, tau, pxp):
    """Build per-core input maps: weights + transposed feature matrix."""
    Dr = np.asarray(D_re, dtype=np.float32)
    Di = np.asarray(D_im, dtype=np.float32)
    tau = np.asarray(tau, dtype=np.float32)

    Wr = Dr[:, None, :] * Dr[None, :, :] + Di[:, None, :] * Di[None, :, :]
    Wi = Di[:, None, :] * Dr[None, :, :] - Dr[:, None, :] * Di[None, :, :]
    W = np.empty((K_FEAT, pxp), dtype=np.float32)
    W[:, N_PX:] = 0.0
    W[:16, :N_PX] = Wr.reshape(16, N_PX)
    W[16:32, :N_PX] = Wi.reshape(16, N_PX)
    W[32, :N_PX] = -tau

    X = np.empty((K_FEAT, N_SAMPLES), dtype=np.float32)
    X[:16] = np.asarray(S_re, dtype=np.float32).reshape(N_SAMPLES, 16).T
    X[16:32] = np.asarray(S_im, dtype=np.float32).reshape(N_SAMPLES, 16).T
    X[32] = 1.0

    in_maps = []
    for i in range(N_CORES):
        xtw = np.empty((K_FEAT, pxp + N_LOC), dtype=np.float32)
        xtw[:, :pxp] = W
        xtw[:, pxp:] = X[:, i * N_LOC : (i + 1) * N_LOC]
        in_maps.append({"xTw": xtw})
    return in_maps


def _features(S_re, S_im, D_re, D_im, tau):
    """(32, N) feature matrix X32 and (32, 242) weight matrix W32 (fp32)."""
    Dr = np.asarray(D_re, dtype=np.float32)
    Di = np.asarray(D_im, dtype=np.float32)
    Wr = Dr[:, None, :] * Dr[None, :, :] + Di[:, None, :] * Di[None, :, :]
    Wi = Di[:, None, :] * Dr[None, :, :] - Dr[:, None, :] * Di[None, :, :]
    W32 = np.concatenate([Wr.reshape(16, N_PX), Wi.reshape(16, N_PX)], 0)
    X32 = np.empty((32, N_SAMPLES), dtype=np.float32)
    X32[:16] = np.asarray(S_re, dtype=np.float32).reshape(N_SAMPLES, 16).T
    X32[16:] = np.asarray(S_im, dtype=np.float32).reshape(N_SAMPLES, 16).T
    return X32, W32


def _bf16_planes(a):
    import ml_dtypes

    bf = ml_dtypes.bfloat16
    h = a.astype(bf)
    r = a - h.astype(np.float32)
    m = r.astype(bf)
    l = (r - m.astype(np.float32)).astype(bf)
    return h, m, l


def _features16(S_re, S_im, D_re, D_im, tau):
    """(16, N) reduced features and (16, 242) weights using Hermitian
    symmetry: 4 diagonal + 6 sym-offdiag (weight doubled) + 6 antisym-
    offdiag (weight doubled)."""
    Dr = np.asarray(D_re, dtype=np.float32)
    Di = np.asarray(D_im, dtype=np.float32)
    Wr = (Dr[:, None, :] * Dr[None, :, :] + Di[:, None, :] * Di[None, :, :]).reshape(
        16, N_PX
    )
    Wi = (Di[:, None, :] * Dr[None, :, :] - Dr[:, None, :] * Di[None, :, :]).reshape(
        16, N_PX
    )
    W16 = np.concatenate([Wr[IDX_DIAG], 2.0 * Wr[IDX_OFF], 2.0 * Wi[IDX_OFF]], 0)

    Sr = np.asarray(S_re, dtype=np.float32).reshape(N_SAMPLES, 16)
    Si = np.asarray(S_im, dtype=np.float32).reshape(N_SAMPLES, 16)
    X16 = np.empty((16, N_SAMPLES), dtype=np.float32)
    X16[0:4] = Sr[:, IDX_DIAG].T
    X16[4:10] = Sr[:, IDX_OFF].T
    X16[10:16] = Si[:, IDX_OFF].T
    return X16, W16


def _pack_host_b(S_re, S_im, D_re, D_im, tau):
    import ml_dtypes

    bf = ml_dtypes.bfloat16
    X16, W16 = _features16(S_re, S_im, D_re, D_im, tau)
    Xh, Xm, Xl = _bf16_planes(X16)
    Wh, Wm, Wl = _bf16_planes(W16)

    # K-stacked pairs: hh, mh, hm, lh, hl, mm
    wA = np.concatenate([Wh, Wh, Wm, Wh, Wl, Wm], 0)  # (96, 242)
    xstack = np.concatenate([Xh, Xm, Xh, Xl, Xh, Xm], 0)  # (96, N)

    taus = np.zeros((128, 2), dtype=np.float32)
    tau = np.asarray(tau, dtype=np.float32)
    taus[: PX_CHUNKS[0], 0] = -tau[: PX_CHUNKS[0]]
    taus[: PX_CHUNKS[1], 1] = -tau[PX_CHUNKS[0] :]

    in_maps = []
    for i in range(N_CORES):
        cols = np.empty((96, W_PAD + N_LOC), dtype=bf)
        cols[:, :W_PAD] = wA
        cols[:, W_PAD:] = xstack[:, i * N_LOC : (i + 1) * N_LOC]
        in_maps.append({"xw": cols, "taus": taus})
    return in_maps


def _pack_host_a16(S_re, S_im, D_re, D_im, tau, pmajor=False):
    import ml_dtypes

    bf = ml_dtypes.bfloat16
    X16, W16 = _features16(S_re, S_im, D_re, D_im, tau)
    Xh, Xm, Xl = _bf16_planes(X16)
    Wh, Wm, Wl = _bf16_planes(W16)
    th, tm, tl = _bf16_planes(-np.asarray(tau, dtype=np.float32)[None, :])

    ones = np.ones((1, N_SAMPLES), dtype=np.float32).astype(bf)
    xstack = np.concatenate([Xh, Xm, Xh, Xl, Xh, Xm, ones, ones, ones], 0)  # (99, N)
    wstack = np.concatenate([Wh, Wh, Wm, Wh, Wl, Wm, th, tm, tl], 0)        # (99, 242)

    in_maps = []
    for i in range(N_CORES):
        xcore = xstack[:, i * N_LOC : (i + 1) * N_LOC]
        if pmajor:
            # device tile t partition p holds sample p*64 + t
            xcore = np.ascontiguousarray(
                xcore.reshape(99, TILE, N_TILES).swapaxes(1, 2).reshape(99, N_LOC)
            )
        cols = np.empty((99, N_PX + N_LOC), dtype=bf)
        cols[:, :N_PX] = wstack
        cols[:, N_PX:] = xcore
        in_maps.append({"xTw": cols})
    return in_maps


def _run(inputs, trace=False, cfg=None):
    from concourse.bass_utils import run_bass_kernel_spmd

    nc, full_cfg = _get_nc(cfg)
    if full_cfg["layout"] == "B":
        in_maps = _pack_host_b(**inputs)
        res = run_bass_kernel_spmd(nc, in_maps, list(range(N_CORES)), trace=trace)
        out = np.empty((N_SAMPLES, N_PX), dtype=np.float32)
        n_groups = N_LOC // full_cfg["group_w"]
        for i in range(N_CORES):
            if full_cfg["linear_out"]:
                buf = res.results[i]["out_flat"]
                outT = np.empty((N_PX, N_LOC), dtype=np.float32)
                off = 0
                px_lo = 0
                for px_w in PX_CHUNKS:
                    gw = full_cfg["group_w"]
                    for g in range(n_groups):
                        blk = buf[off : off + px_w * gw].reshape(px_w, gw)
                        outT[px_lo : px_lo + px_w, g * gw : (g + 1) * gw] = blk
                        off += px_w * gw
                    px_lo += px_w
            else:
                outT = res.results[i]["outT"]
            out[i * N_LOC : (i + 1) * N_LOC] = outT.T
    elif full_cfg.get("bf16split"):
        in_maps = _pack_host_a16(**inputs, pmajor=full_cfg.get("pmajor", False))
        res = run_bass_kernel_spmd(nc, in_maps, list(range(N_CORES)), trace=trace)
        out = np.concatenate(
            [res.results[i]["out"] for i in range(N_CORES)], axis=0
        )
        return out, res
    else:
        pxp = 256 if full_cfg["f32r"] else N_PX
        in_maps = _pack_host(**inputs, pxp=pxp)
        res = run_bass_kernel_spmd(nc, in_maps, list(range(N_CORES)), trace=trace)
        out = np.concatenate(
            [res.results[i]["out"] for i in range(N_CORES)], axis=0
        )
    return out, res


def kernel(**inputs) -> np.ndarray:
    out, _ = _run(inputs, trace=False)
    return out


# revision 46
# speedup vs baseline: 1.0038x; 1.0038x over previous
"""Trainium2 Bass kernel for nn_BackProjLayer.

Math: the reference computes, per sample n,
    eigh(S) -> (lam, V);  G = V @ diag(sqrt(max(lam,0)));  y = D^H G
    out[n,p] = sum_d |y[p,d]|^2 - tau[p] = [D^H S_plus D]_pp - tau[p]
Since S = A A^H / Nch is Hermitian PSD by construction, S_plus == S up to
float32 eigensolver noise, so no eigendecomposition is needed:
    out[n,p] = Re(d_p^H S[n] d_p) - tau[p]
With S = Sr + i Si (Sr sym, Si antisym) and d = dr + i di this is a real
bilinear form; by Hermitian symmetry it reduces to 16 features per sample
(4 diag Sr, 6 offdiag Sr doubled, 6 offdiag Si doubled):
    out = X16.T @ W16 - tau     X16 (16, N), W16 (16, 242)

Device kernel (default config, layout A + bf16split):
  - exact-fp32 matmul via a K-stacked bf16 3-way split: X16 = Xh+Xm+Xl,
    W16 = Wh+Wm+Wl (bf16 planes); the six significant cross-terms
    (hh, mh, hm, lh, hl, mm) plus three const-one rows against the -tau
    bf16 planes are stacked into ONE K=99 bf16 matmul per 128-sample tile
    (dropped ml/lm/ll terms are ~2^-27 relative).
  - per 128-sample tile: PSUM[128,242] = lhsT(99,128).T @ Wstack(99,242);
    PSUM->SBUF copies alternate scalar/vector engines; 8 grouped ~1MB
    output DMAs alternate the two HWDGE queues (sync/scalar).
  - built on Bacc + TileContext: Bacc.compile() splits multi-wait
    sync_infos into chained EventSemaphores (TRN2 ISA allows only one
    sync wait per instruction).

Sharding: pure data parallel over N across 8 cores (8192 samples/core);
host packs per-core inputs, device returns (8192, 242) per core,
host concatenates.
"""

import sys

for _p in ("/opt/trn_rl_repo", "/root/.axon_site/_ro/trn_rl_repo"):
    if _p not in sys.path:
        sys.path.insert(0, _p)

import numpy as np

N_SAMPLES = 65536
N_CH = 4
N_PX = 242
N_CORES = 8
N_LOC = N_SAMPLES // N_CORES  # 8192

K_FEAT = 2 * N_CH * N_CH + 1  # 33

TILE = 128
N_TILES = N_LOC // TILE  # 64

DEFAULT_CFG = dict(
    layout="A",      # A: samples-on-partitions; B: pixels-on-partitions
    bf16split=True,  # exact-fp32 via K-stacked bf16 3-way split (K=99)
    f32r=True,       # (non-split A only) float32r matmul
    pack=2,          # matmul outputs per PSUM tile / per copy instruction
    psum_bufs=4,     # PSUM pool slots
    groups=8,        # number of output DMAs
    dma_engines=("sync", "scalar"),  # round-robin for output DMAs
    copy_pattern="vvs",  # per-copy engine cycle: s=scalar, v=vector
    x_chunks=8,      # input DMA chunks
    slice_w=512,     # B: samples per matmul (moving dim)
    group_w=2048,    # B: samples per output DMA group
    in_eng="sync",   # engine issuing input DMAs
    linear_out=True, # B: write output groups as contiguous HBM blocks
    pmajor=True,     # A: partition-major sample mapping (contiguous 7.7KB
                     # HBM runs per partition in output DMAs)
    tail="sem_only", # lighter Tile exit: sem-only barrier around sem clears
)

# B layout constants
PX_CHUNKS = (128, 114)  # pixels per chunk (DMA uses 8 partitions/SDMA engine,
                        # so a 128-partition transfer engages all 16 engines)
W_PAD = 242          # leading cols of xw holding the stacked W planes
IDX_DIAG = [0, 5, 10, 15]       # S[c,c] positions in the c*4+c' flattening
IDX_OFF = [1, 2, 3, 6, 7, 11]   # S[c,c'] c<c' positions

_BUILT = {}


def _tc_class(base, cfg):
    """Optionally lighten the Tile kernel-tail: keep the drain (output DMA
    completion) and the semaphore clears (needed for NEFF re-execution),
    but trim barrier work per cfg['tail'] mode."""
    mode = cfg.get("tail", "full")
    if mode == "full":
        return base

    from concourse.vector_clock import ScopedClock

    class _TC(base):
        def _drain_and_barrier(self, tick_clock, wait_clock):
            nc = self.nc
            drain_inst = nc.sync.drain()
            wait_clock.add_sem_waits(
                drain_inst.ins, ScopedClock({None: tick_clock.global_clock})
            )
            if mode == "sem_only":
                nc.all_engine_barrier(sem_only=True)
            else:
                nc.all_engine_barrier()
            popped = nc._tile_sem_poison_stack.pop()
            assert popped is self._sem_poison
            nc.clear_and_free_semaphores(list(self.sems.allocated().values()))
            if mode not in ("no2nd", "sem_only"):
                nc.all_engine_barrier()

    return _TC


def _build_nc(cfg):
    import concourse.mybir as mybir
    from concourse import bacc
    from concourse.tile import TileContext

    f32 = mybir.dt.float32
    f32r = mybir.dt.float32r
    bf16 = mybir.dt.bfloat16

    if cfg.get("bf16split"):
        # exact-fp32 bf16 3-way split (see _build_nc_b docstring), with two
        # extra const-one rows pairing against the -tau bf16 planes: K=99
        pxp = N_PX
        in_dt = bf16
        kf = 99
    else:
        pxp = 256 if cfg["f32r"] else N_PX
        in_dt = f32r if cfg["f32r"] else f32
        kf = K_FEAT
    ps_stride = 256
    pack = cfg["pack"]
    groups = cfg["groups"]
    tiles_per_group = N_TILES // groups
    assert tiles_per_group % pack == 0 or pack % tiles_per_group == 0

    # Bacc (not plain Bass): its compile() lowers multi-wait sync_infos into
    # chained EventSemaphores (TRN2 allows 1 wait/instruction).
    nc = bacc.Bacc("TRN2", target_bir_lowering=False, debug=False)
    TileContext = _tc_class(TileContext, cfg)
    xTw = nc.declare_dram_parameter("xTw", [kf, pxp + N_LOC], in_dt, isOutput=False)
    out = nc.declare_dram_parameter("out", [N_LOC, N_PX], f32, isOutput=True)

    if cfg.get("pmajor"):
        # partition-major sample mapping: tile t, partition p <-> sample
        # n = p*64 + t. Each partition's 8-tile group lands in 8
        # CONSECUTIVE output rows -> 7744B contiguous HBM runs per
        # partition (vs 968B strided), much better SDMA descriptor
        # efficiency. Host permutes the input columns to match.
        out_g = out.rearrange(
            "(p g j) c -> g p (j c)", p=TILE, g=groups, j=tiles_per_group
        )
    else:
        out_g = out.rearrange("(g j p) c -> g p j c", p=TILE, j=tiles_per_group)
    x_chunk = N_LOC // cfg["x_chunks"]

    with TileContext(nc) as tc:
        with (
            tc.tile_pool(name="xin", bufs=1) as xpool,
            tc.tile_pool(name="ps", bufs=cfg["psum_bufs"], space="PSUM") as pspool,
            tc.tile_pool(name="ob", bufs=1) as opool,
        ):
            if cfg.get("sep_in"):
                in_eng = getattr(nc, cfg["in_eng"])
                wt_tile = xpool.tile([kf, pxp], in_dt, tag="w")
                in_eng.dma_start(wt_tile[:], xTw[:, :pxp])
                wt = wt_tile[:]
                xts = []
                for ci in range(cfg["x_chunks"]):
                    xt = xpool.tile([kf, x_chunk], in_dt, tag=f"x{ci}")
                    lo = pxp + ci * x_chunk
                    in_eng.dma_start(xt[:], xTw[:, lo : lo + x_chunk])
                    xts.append(xt)

                def lhs_ap(t):
                    ci, off = divmod(t * TILE, x_chunk)
                    return xts[ci][:, off : off + TILE]
            else:
                in_eng = getattr(nc, cfg["in_eng"])
                xt0 = xpool.tile([kf, pxp + N_LOC], in_dt)
                in_eng.dma_start(xt0[:, : pxp + x_chunk], xTw[:, : pxp + x_chunk])
                for ci in range(1, cfg["x_chunks"]):
                    lo = pxp + ci * x_chunk
                    in_eng.dma_start(xt0[:, lo : lo + x_chunk], xTw[:, lo : lo + x_chunk])
                wt = xt0[:, :pxp]

                def lhs_ap(t):
                    off = pxp + t * TILE
                    return xt0[:, off : off + TILE]

            copy_engines = {
                "s": nc.scalar.copy,
                "v": nc.vector.tensor_copy,
            }
            dma_engines = [getattr(nc, e) for e in cfg["dma_engines"]]

            copy_idx = 0
            for g in range(groups):
                gt = opool.tile([TILE, tiles_per_group * N_PX], f32, tag=f"g{g}")
                for jp in range(tiles_per_group // pack):
                    ps = pspool.tile([TILE, pack * ps_stride], f32)
                    for h in range(pack):
                        t = g * tiles_per_group + jp * pack + h
                        nc.tensor.matmul(
                            ps[:, h * ps_stride : h * ps_stride + pxp],
                            lhs_ap(t),
                            wt,
                            start=True,
                            stop=True,
                        )
                    src = ps[:].rearrange("p (h c) -> p h c", h=pack)[:, :, :N_PX]
                    lo = jp * pack * N_PX
                    dst = gt[:, lo : lo + pack * N_PX].rearrange(
                        "p (h c) -> p h c", h=pack
                    )
                    pat = cfg["copy_pattern"]
                    copy_engines[pat[copy_idx % len(pat)]](dst, src)
                    copy_idx += 1
                if cfg.get("pmajor") and cfg.get("split_dma"):
                    # both HWDGE queues stream halves of the same group
                    # concurrently: halves per-queue blocking time
                    dma_engines[0].dma_start(out_g[g][:64], gt[:64])
                    dma_engines[1].dma_start(out_g[g][64:], gt[64:])
                elif cfg.get("pmajor"):
                    dma_engines[g % len(dma_engines)].dma_start(out_g[g], gt[:])
                else:
                    dma_engines[g % len(dma_engines)].dma_start(
                        out_g[g],
                        gt[:].rearrange("p (j c) -> p j c", j=tiles_per_group),
                    )

    nc.compile()
    return nc


def _build_nc_b(cfg):
    """Pixels-on-partitions layout with exact-fp32 bf16 3-way split.

    The bilinear form is reduced to 16 features per sample via the
    Hermitian symmetry of S (4 diag + 6 sym-offdiag + 6 antisym-offdiag,
    off-diagonal weights doubled). X16 = Xh+Xm+Xl, W16 = Wh+Wm+Wl (bf16
    planes); the six significant cross-terms (hh, mh, hm, lh, hl, mm) are
    K-stacked into ONE bf16 matmul with K = 6*16 = 96:
      [Xh;Xm;Xh;Xl;Xh;Xm] x [Wh;Wh;Wm;Wh;Wl;Wm]
    (dropped ml/lm/ll terms are ~2^-27 relative -> fp32-grade accuracy).
    Stationary operand is the W side (reloaded only on pixel-chunk switch),
    moving is samples, so the PE streams at 1 col/cycle. The output lands
    transposed (242 x 8192) and is unscrambled on the host. tau is applied
    per-partition during the PSUM->SBUF copy (ACT Identity bias / DVE
    tensor_scalar add). Pixel chunks are 128 + 114 so output DMAs engage
    all 16 SDMA engines (8 partitions each).
    """
    import concourse.mybir as mybir
    from concourse import bacc
    from concourse.tile import TileContext

    f32 = mybir.dt.float32
    bf16 = mybir.dt.bfloat16

    slice_w = cfg["slice_w"]
    group_w = cfg["group_w"]
    slices_per_group = group_w // slice_w
    n_groups = N_LOC // group_w

    nc = bacc.Bacc("TRN2", target_bir_lowering=False, debug=False)
    TileContext = _tc_class(TileContext, cfg)
    xw = nc.declare_dram_parameter("xw", [96, W_PAD + N_LOC], bf16, isOutput=False)
    taus = nc.declare_dram_parameter("taus", [128, 2], f32, isOutput=False)
    if cfg["linear_out"]:
        # each (px_w, group_w) staging tile lands as one contiguous HBM
        # block -> the DMA splits evenly across all 16 SDMA engines
        out_flat = nc.declare_dram_parameter("out_flat", [N_PX * N_LOC], f32, isOutput=True)
    else:
        outT = nc.declare_dram_parameter("outT", [N_PX, N_LOC], f32, isOutput=True)

    n_xchunks = cfg["x_chunks"]
    x_chunk = N_LOC // n_xchunks

    with TileContext(nc) as tc:
        with (
            tc.tile_pool(name="xin", bufs=1) as xpool,
            tc.tile_pool(name="ps", bufs=cfg["psum_bufs"], space="PSUM") as pspool,
            tc.tile_pool(name="ob", bufs=1) as opool,
        ):
            # W planes + tau first (small, fast), then per-chunk sample
            # tiles so early matmuls only wait on their own chunk's DMA
            in_eng = getattr(nc, cfg["in_eng"])
            wtile = xpool.tile([96, W_PAD], bf16, tag="w")
            in_eng.dma_start(wtile[:], xw[:, :W_PAD])
            tt = xpool.tile([128, 2], f32, tag="taus")
            in_eng.dma_start(tt[:], taus[:])
            xts = []
            for ci in range(n_xchunks):
                xt = xpool.tile([96, x_chunk], bf16, tag=f"x{ci}")
                lo = W_PAD + ci * x_chunk
                in_eng.dma_start(xt[:], xw[:, lo : lo + x_chunk])
                xts.append(xt)

            dma_engines = [getattr(nc, e) for e in cfg["dma_engines"]]
            copy_idx = 0
            dma_idx = 0
            px_lo = 0
            for c, px_w in enumerate(PX_CHUNKS):
                wa = wtile[:, px_lo : px_lo + px_w]
                for g in range(n_groups):
                    gt = opool.tile([px_w, group_w], f32, tag=f"g{c}_{g}")
                    for sj in range(slices_per_group):
                        s = g * slices_per_group + sj
                        ci, off = divmod(s * slice_w, x_chunk)
                        xm = xts[ci][:, off : off + slice_w]
                        ps = pspool.tile([px_w, slice_w], f32, tag="ps")
                        nc.tensor.matmul(ps[:], wa, xm, start=True, stop=True)
                        dst = gt[:, sj * slice_w : (sj + 1) * slice_w]
                        pat = cfg["copy_pattern"]
                        eng = pat[copy_idx % len(pat)]
                        if eng == "s":
                            nc.scalar.activation(
                                dst,
                                ps[:],
                                mybir.ActivationFunctionType.Identity,
                                bias=tt[:px_w, c : c + 1],
                            )
                        else:
                            nc.vector.tensor_scalar_add(
                                dst, ps[:], tt[:px_w, c : c + 1]
                            )
                        copy_idx += 1
                    if cfg["linear_out"]:
                        off = (px_lo * N_LOC) + g * px_w * group_w
                        dest = out_flat[off : off + px_w * group_w].rearrange(
                            "(p c) -> p c", p=px_w
                        )
                    else:
                        dest = outT[
                            px_lo : px_lo + px_w, g * group_w : (g + 1) * group_w
                        ]
                    dma_engines[dma_idx % len(dma_engines)].dma_start(dest, gt[:])
                    dma_idx += 1
                px_lo += px_w

    nc.compile()
    return nc


def _get_nc(cfg=None):
    cfg = dict(DEFAULT_CFG, **(cfg or {}))
    key = tuple(sorted((k, str(v)) for k, v in cfg.items()))
    if key not in _BUILT:
        builder = _build_nc_b if cfg["layout"] == "B" else _build_nc
        _BUILT[key] = (builder(cfg), cfg)
    return _BUILT[key]


def _pack_host(S_re, S_im, D_re, D_im, tau, pxp):
    """Build per-core input maps: weights + transposed feature matrix."""
    Dr = np.asarray(D_re, dtype=np.float32)
    Di = np.asarray(D_im, dtype=np.float32)
    tau = np.asarray(tau, dtype=np.float32)

    Wr = Dr[:, None, :] * Dr[None, :, :] + Di[:, None, :] * Di[None, :, :]
    Wi = Di[:, None, :] * Dr[None, :, :] - Dr[:, None, :] * Di[None, :, :]
    W = np.empty((K_FEAT, pxp), dtype=np.float32)
    W[:, N_PX:] = 0.0
    W[:16, :N_PX] = Wr.reshape(16, N_PX)
    W[16:32, :N_PX] = Wi.reshape(16, N_PX)
    W[32, :N_PX] = -tau

    X = np.empty((K_FEAT, N_SAMPLES), dtype=np.float32)
    X[:16] = np.asarray(S_re, dtype=np.float32).reshape(N_SAMPLES, 16).T
    X[16:32] = np.asarray(S_im, dtype=np.float32).reshape(N_SAMPLES, 16).T
    X[32] = 1.0

    in_maps = []
    for i in range(N_CORES):
        xtw = np.empty((K_FEAT, pxp + N_LOC), dtype=np.float32)
        xtw[:, :pxp] = W
        xtw[:, pxp:] = X[:, i * N_LOC : (i + 1) * N_LOC]
        in_maps.append({"xTw": xtw})
    return in_maps


def _features(S_re, S_im, D_re, D_im, tau):
    """(32, N) feature matrix X32 and (32, 242) weight matrix W32 (fp32)."""
    Dr = np.asarray(D_re, dtype=np.float32)
    Di = np.asarray(D_im, dtype=np.float32)
    Wr = Dr[:, None, :] * Dr[None, :, :] + Di[:, None, :] * Di[None, :, :]
    Wi = Di[:, None, :] * Dr[None, :, :] - Dr[:, None, :] * Di[None, :, :]
    W32 = np.concatenate([Wr.reshape(16, N_PX), Wi.reshape(16, N_PX)], 0)
    X32 = np.empty((32, N_SAMPLES), dtype=np.float32)
    X32[:16] = np.asarray(S_re, dtype=np.float32).reshape(N_SAMPLES, 16).T
    X32[16:] = np.asarray(S_im, dtype=np.float32).reshape(N_SAMPLES, 16).T
    return X32, W32


def _bf16_planes(a):
    import ml_dtypes

    bf = ml_dtypes.bfloat16
    h = a.astype(bf)
    r = a - h.astype(np.float32)
    m = r.astype(bf)
    l = (r - m.astype(np.float32)).astype(bf)
    return h, m, l


def _features16(S_re, S_im, D_re, D_im, tau):
    """(16, N) reduced features and (16, 242) weights using Hermitian
    symmetry: 4 diagonal + 6 sym-offdiag (weight doubled) + 6 antisym-
    offdiag (weight doubled)."""
    Dr = np.asarray(D_re, dtype=np.float32)
    Di = np.asarray(D_im, dtype=np.float32)
    Wr = (Dr[:, None, :] * Dr[None, :, :] + Di[:, None, :] * Di[None, :, :]).reshape(
        16, N_PX
    )
    Wi = (Di[:, None, :] * Dr[None, :, :] - Dr[:, None, :] * Di[None, :, :]).reshape(
        16, N_PX
    )
    W16 = np.concatenate([Wr[IDX_DIAG], 2.0 * Wr[IDX_OFF], 2.0 * Wi[IDX_OFF]], 0)

    Sr = np.asarray(S_re, dtype=np.float32).reshape(N_SAMPLES, 16)
    Si = np.asarray(S_im, dtype=np.float32).reshape(N_SAMPLES, 16)
    X16 = np.empty((16, N_SAMPLES), dtype=np.float32)
    X16[0:4] = Sr[:, IDX_DIAG].T
    X16[4:10] = Sr[:, IDX_OFF].T
    X16[10:16] = Si[:, IDX_OFF].T
    return X16, W16


def _pack_host_b(S_re, S_im, D_re, D_im, tau):
    import ml_dtypes

    bf = ml_dtypes.bfloat16
    X16, W16 = _features16(S_re, S_im, D_re, D_im, tau)
    Xh, Xm, Xl = _bf16_planes(X16)
    Wh, Wm, Wl = _bf16_planes(W16)

    # K-stacked pairs: hh, mh, hm, lh, hl, mm
    wA = np.concatenate([Wh, Wh, Wm, Wh, Wl, Wm], 0)  # (96, 242)
    xstack = np.concatenate([Xh, Xm, Xh, Xl, Xh, Xm], 0)  # (96, N)

    taus = np.zeros((128, 2), dtype=np.float32)
    tau = np.asarray(tau, dtype=np.float32)
    taus[: PX_CHUNKS[0], 0] = -tau[: PX_CHUNKS[0]]
    taus[: PX_CHUNKS[1], 1] = -tau[PX_CHUNKS[0] :]

    in_maps = []
    for i in range(N_CORES):
        cols = np.empty((96, W_PAD + N_LOC), dtype=bf)
        cols[:, :W_PAD] = wA
        cols[:, W_PAD:] = xstack[:, i * N_LOC : (i + 1) * N_LOC]
        in_maps.append({"xw": cols, "taus": taus})
    return in_maps


def _pack_host_a16(S_re, S_im, D_re, D_im, tau, pmajor=False):
    import ml_dtypes

    bf = ml_dtypes.bfloat16
    X16, W16 = _features16(S_re, S_im, D_re, D_im, tau)
    Xh, Xm, Xl = _bf16_planes(X16)
    Wh, Wm, Wl = _bf16_planes(W16)
    th, tm, tl = _bf16_planes(-np.asarray(tau, dtype=np.float32)[None, :])

    ones = np.ones((1, N_SAMPLES), dtype=np.float32).astype(bf)
    xstack = np.concatenate([Xh, Xm, Xh, Xl, Xh, Xm, ones, ones, ones], 0)  # (99, N)
    wstack = np.concatenate([Wh, Wh, Wm, Wh, Wl, Wm, th, tm, tl], 0)        # (99, 242)

    in_maps = []
    for i in range(N_CORES):
        xcore = xstack[:, i * N_LOC : (i + 1) * N_LOC]
        if pmajor:
            # device tile t partition p holds sample p*64 + t
            xcore = np.ascontiguousarray(
                xcore.reshape(99, TILE, N_TILES).swapaxes(1, 2).reshape(99, N_LOC)
            )
        cols = np.empty((99, N_PX + N_LOC), dtype=bf)
        cols[:, :N_PX] = wstack
        cols[:, N_PX:] = xcore
        in_maps.append({"xTw": cols})
    return in_maps


def _run(inputs, trace=False, cfg=None):
    from concourse.bass_utils import run_bass_kernel_spmd

    nc, full_cfg = _get_nc(cfg)
    if full_cfg["layout"] == "B":
        in_maps = _pack_host_b(**inputs)
        res = run_bass_kernel_spmd(nc, in_maps, list(range(N_CORES)), trace=trace)
        out = np.empty((N_SAMPLES, N_PX), dtype=np.float32)
        n_groups = N_LOC // full_cfg["group_w"]
        for i in range(N_CORES):
            if full_cfg["linear_out"]:
                buf = res.results[i]["out_flat"]
                outT = np.empty((N_PX, N_LOC), dtype=np.float32)
                off = 0
                px_lo = 0
                for px_w in PX_CHUNKS:
                    gw = full_cfg["group_w"]
                    for g in range(n_groups):
                        blk = buf[off : off + px_w * gw].reshape(px_w, gw)
                        outT[px_lo : px_lo + px_w, g * gw : (g + 1) * gw] = blk
                        off += px_w * gw
                    px_lo += px_w
            else:
                outT = res.results[i]["outT"]
            out[i * N_LOC : (i + 1) * N_LOC] = outT.T
    elif full_cfg.get("bf16split"):
        in_maps = _pack_host_a16(**inputs, pmajor=full_cfg.get("pmajor", False))
        res = run_bass_kernel_spmd(nc, in_maps, list(range(N_CORES)), trace=trace)
        out = np.concatenate(
            [res.results[i]["out"] for i in range(N_CORES)], axis=0
        )
        return out, res
    else:
        pxp = 256 if full_cfg["f32r"] else N_PX
        in_maps = _pack_host(**inputs, pxp=pxp)
        res = run_bass_kernel_spmd(nc, in_maps, list(range(N_CORES)), trace=trace)
        out = np.concatenate(
            [res.results[i]["out"] for i in range(N_CORES)], axis=0
        )
    return out, res


def kernel(**inputs) -> np.ndarray:
    out, _ = _run(inputs, trace=False)
    return out
